# revision 2
# baseline (speedup 1.0000x reference)
"""Trainium2 Bass kernel for nn_CrossCorrelation (v2).

Per core (one batch of 8): c=32 channels of 128x128.
  xs = standardize(x); Xf = fft2(xs); for ordered pairs (i, j>=i):
  cc = real(ifft2(Xf_i * conj(Xf_j))) rolled by (10,10), windowed 21x21.

v2 layout (vs baseline):
  - stage A y-FFT: single f32r matmul per channel (x f32r stationary,
    [Fr|Fi] f32r moving, 256 cols -> full rate). No bf16 hi/lo splits.
  - stats: per-chunk (8ch) DVE X-reduces + Pool C-reduce + tiny chain;
    descending chunk order so the pair pipeline starts early.
  - Gauss product planes m1/m2/m3 split across DVE and Pool.
  - D matmuls into 2-bank PSUM supertiles; one Act copy per 2 banks.
  - 42-row quadrant transposes (out free 42/pair instead of 64/128).
  - out accumulated in SBUF [21, 528, 21]; 2 DMAs total; host un-permutes
    emission slots back to reference pair order.
"""

import os
import numpy as np


class _DebugDone(Exception):
    def __init__(self, nc):
        self.nc = nc

DEBUG_STOP = int(os.environ.get("K_DEBUG_STOP", "0"))  # 1=phase1 only

H = W = 128
C = 32
B = 8
NPIX = H * W
MAX_S = 10
S = 2 * MAX_S + 1  # 21
NPAIR = C * (C + 1) // 2  # 528
STD_EPS = 1e-9
UPAD = 66
NU = 65

CHUNKS = [(24, 32), (16, 24), (8, 16), (0, 8)]
GROUPS = [(28, 32), (21, 28), (14, 21), (7, 14), (0, 7)]


def emission_plan():
    """Mirror of the phase-2 emission loop. Returns list of banks;
    each bank is (subA, subB or None) with sub = (i, s0, w, slot).
    Slots are assigned in emission order: dual banks place subA's w pairs
    then subB's w pairs contiguously."""
    banks = []
    slot = 0
    pend = None
    for i in range(C - 1, -1, -1):
        npairs = C - i
        for s0 in range(0, npairs, 7):
            w = min(7, npairs - s0)
            if w == 7:
                if pend is None:
                    pend = (i, s0)
                else:
                    a = (pend[0], pend[1], 7, slot)
                    b = (i, s0, 7, slot + 7)
                    banks.append((a, b))
                    slot += 14
                    pend = None
            else:
                banks.append(((i, s0, w, slot), None))
                slot += w
    if pend is not None:
        banks.append(((pend[0], pend[1], 7, slot), None))
        slot += 7
    assert slot == NPAIR
    return banks


def slot_to_pair():
    """slot index -> reference pair index (row-major i ascending)."""
    m = np.zeros(NPAIR, dtype=np.int64)
    for subA, subB in emission_plan():
        for sub in (subA, subB):
            if sub is None:
                continue
            i, s0, w, slot = sub
            p_start = i * C - i * (i - 1) // 2  # pairs before row i
            for t in range(w):
                m[slot + t] = p_start + s0 + t
    return m


def _host_constants():
    import ml_dtypes

    k = np.arange(H)
    F = np.exp(-2j * np.pi * np.outer(k, k) / H)
    Fr = np.ascontiguousarray(F.real, np.float32)
    Fi = np.ascontiguousarray(F.imag, np.float32)

    # stage-A moving operand (f32r): [Fr | Fi]
    fmov = np.concatenate([Fr, Fi], axis=1).astype(np.float32)  # (128, 256)

    # stage-B stationaries: Fr, Fi, -Fi (bf16)
    fmats = np.concatenate([Fr, Fi, -Fi], axis=1)  # (128, 384)

    sy = (np.arange(S) - MAX_S) % H
    u = np.arange(NU)
    Gy = np.exp(2j * np.pi * np.outer(sy, u) / H)
    w_u = np.ones(NU)
    w_u[1:64] = 2.0
    Gyw = Gy * w_u
    Gx = np.exp(2j * np.pi * np.outer(sy, np.arange(W)) / W) / NPIX

    Gxr = Gx.real.astype(np.float32)
    Gxi = Gx.imag.astype(np.float32)
    S1 = np.concatenate([Gxr, Gxi], axis=0)  # (42, 128)
    S2 = np.concatenate([-Gxi, Gxr], axis=0)
    S12 = S1 - S2
    pad = np.zeros((22, 128), np.float32)
    smats = np.concatenate(
        [np.concatenate([Sm, pad], axis=0).T for Sm in (S1, S12, S2)],
        axis=1)  # (128, 192)

    Gywr = Gyw.real.astype(np.float32)
    Gywi = Gyw.imag.astype(np.float32)
    gys = np.zeros((128, 42), np.float32)
    gys[0:NU, 0:21] = Gywr.T
    gys[0:NU, 21:42] = -Gywi.T

    id128 = np.eye(128, dtype=np.float32)

    # one bf16 constant blob: [fmats(384) | smats(192) | gys(42) | id128(128)]
    blob = np.concatenate([fmats, smats, gys, id128], axis=1)  # (128, 746)
    blob = blob.astype(ml_dtypes.bfloat16)

    ones_row = np.ones((1, 128), np.float32)

    return dict(cb=blob, fmov=fmov, ones_row=ones_row)


def build_nc():
    import concourse.bass as bass
    import concourse.mybir as mybir
    import concourse.tile as tile
    from concourse import bacc
    from contextlib import ExitStack

    f32 = mybir.dt.float32
    f32r = mybir.dt.float32r
    bf16 = mybir.dt.bfloat16
    AF = mybir.ActivationFunctionType
    ALU = mybir.AluOpType

    nc = bacc.Bacc("TRN2", target_bir_lowering=False, debug=False)

    x_d = nc.dram_tensor("x", [C, H, W], f32r, kind="ExternalInput").ap()
    cb_d = nc.dram_tensor("cb", [128, 746], bf16, kind="ExternalInput").ap()
    fmov_d = nc.dram_tensor("fmov", [128, 256], f32r, kind="ExternalInput").ap()
    ones_d = nc.dram_tensor("ones_row", [1, 128], f32, kind="ExternalInput").ap()
    out_d = nc.dram_tensor("out", [S, NPAIR, S], f32, kind="ExternalOutput").ap()

    banks = emission_plan()

    if True:
      with tile.TileContext(nc) as tc, ExitStack() as ctx:
        cpool = ctx.enter_context(tc.tile_pool(name="consts", bufs=1))
        spool = ctx.enter_context(tc.tile_pool(name="work", bufs=1))

        # ---- constant + input loads (x chunk0 first; consts on idle queues)
        X = spool.tile([128, C, W], f32r, tag="X")  # partition=y, free=(c,x)
        lo0, hi0 = CHUNKS[0]
        nc.sync.dma_start(X[:, lo0:hi0, :], x_d[lo0:hi0].transpose([1, 0, 2]))
        cb = cpool.tile([128, 746], bf16, tag="cb")
        nc.scalar.dma_start(cb[:, :], cb_d)
        fmats = cb[:, 0:384]
        smats = cb[:, 384:576]
        gys = cb[0:NU, 576:618]
        id128 = cb[:, 618:746]
        Fr = fmats[:, 0:128]
        Fi = fmats[:, 128:256]
        Fin = fmats[:, 256:384]

        fmov = cpool.tile([128, 256], f32r, tag="fmov")
        nc.gpsimd.dma_start(fmov[:, :], fmov_d)
        ones_row = cpool.tile([1, 128], f32, tag="ones_row")
        nc.gpsimd.dma_start(ones_row[:, :], ones_d)

        for lo, hi in CHUNKS[1:]:
            nc.sync.dma_start(X[:, lo:hi, :],
                              x_d[lo:hi].transpose([1, 0, 2]))
        Xf = X[:, :, :].bitcast(f32)

        # ---- persistent SBUF work tensors ----
        T_s = spool.tile([128, C, 2, UPAD], bf16, tag="T")
        P1 = spool.tile([128, C, UPAD], bf16, tag="P1")
        P2 = spool.tile([128, C, UPAD], bf16, tag="P2")
        P3 = spool.tile([128, C, UPAD], bf16, tag="P3")
        P4 = spool.tile([128, C, UPAD], bf16, tag="P4")
        bc = spool.tile([128, 64], f32, tag="bc")
        red = spool.tile([128, 64], f32, tag="red")
        stats = spool.tile([1, 64], f32, tag="stats")
        out_acc = spool.tile([S, NPAIR, S], f32, tag="out_acc")

        for P in (P1, P2, P3, P4):
            nc.gpsimd.memset(P[:, :, 65:66], 0.0)

        n = float(NPIX)

        # =========================== phase 1 ===========================
        # SBUF pools for phase-2 products are opened early so small-i
        # product tiles can be prefetched during phase 1 (bridges the
        # PSUM pool-swap barrier between phases).
        mpool = ctx.enter_context(tc.tile_pool(name="mpool", bufs=5))
        smpool = ctx.enter_context(tc.tile_pool(name="smpool", bufs=8))
        dspool = ctx.enter_context(tc.tile_pool(name="dspool", bufs=4))
        dtpool = ctx.enter_context(tc.tile_pool(name="dtpool", bufs=3))

        m_tiles = {}

        def get_m(i):
            if i in m_tiles:
                return m_tiles[i]
            npairs = C - i
            if npairs <= 8:
                m1 = smpool.tile([128, 8, UPAD], bf16, tag="sm1")
                m2 = smpool.tile([128, 8, UPAD], bf16, tag="sm2")
                m3 = smpool.tile([128, 8, UPAD], bf16, tag="sm3")
            else:
                m1 = mpool.tile([128, C, UPAD], bf16, tag="m1")
                m2 = mpool.tile([128, C, UPAD], bf16, tag="m2")
                m3 = mpool.tile([128, C, UPAD], bf16, tag="m3")
            bshape = [128, npairs, 65]
            nc.vector.tensor_tensor(m1[:, 0:npairs, 0:65],
                                    P1[:, i:i + 1, 0:65].broadcast_to(bshape),
                                    P4[:, i:, 0:65], op=ALU.mult)
            nc.vector.tensor_tensor(m2[:, 0:npairs, 0:65],
                                    P2[:, i:i + 1, 0:65].broadcast_to(bshape),
                                    P3[:, i:, 0:65], op=ALU.mult)
            # Pool (GPSIMD, ~2ns/elem) helps only on the big late blocks;
            # chunked so the first banks of the block aren't gated.
            if i <= 13:
                for c0, c1 in ((0, 14), (14, npairs)):
                    cs = [128, c1 - c0, 65]
                    nc.gpsimd.tensor_tensor(
                        m3[:, c0:c1, 0:65],
                        P3[:, i:i + 1, 0:65].broadcast_to(cs),
                        P2[:, i + c0:i + c1, 0:65], op=ALU.mult)
            else:
                nc.vector.tensor_tensor(
                    m3[:, 0:npairs, 0:65],
                    P3[:, i:i + 1, 0:65].broadcast_to(bshape),
                    P2[:, i:, 0:65], op=ALU.mult)
            mt = (m1, m2, m3)
            m_tiles[i] = mt
            return mt

        with tc.tile_pool(name="psA", bufs=4, space="PSUM") as psA, \
             tc.tile_pool(name="psB", bufs=1, space="PSUM") as psB, \
             tc.tile_pool(name="psS", bufs=1, space="PSUM") as psS, \
             tc.tile_pool(name="sqp", bufs=2) as sqp, \
             tc.tile_pool(name="chp", bufs=4) as chp:

            bc_ps = psS.tile([128, 512], f32, tag="bcps")

            # scale_part(k) (divide -> bc -> stage-A copies -> stage-B
            # group -> small-i product prefetch) runs one chunk late so
            # no engine queue head-of-line blocks on a cross-engine wait.
            GROUPS_AT = {0: [GROUPS[0]], 1: [GROUPS[1]], 2: [GROUPS[2]],
                         3: [GROUPS[3], GROUPS[4]]}
            PREFETCH_AT = {0: range(31, 27, -1), 1: range(27, 23, -1),
                           2: [23, 22], 3: [21, 20]}

            def emit_group(g, ge):
                w = ge - g
                br_raw = psB.tile([128, 512], f32, tag="br")
                bi_raw = psB.tile([128, 512], f32, tag="bi")
                br = br_raw[:, 0:455].rearrange("p (a b) -> p a b", a=7)
                bi = bi_raw[:, 0:455].rearrange("p (a b) -> p a b", a=7)
                TrT = T_s[:, g:ge, 0, 0:65]
                TiT = T_s[:, g:ge, 1, 0:65]
                nc.tensor.matmul(br[:, 0:w, :], Fr, TrT, start=True, stop=False)
                nc.tensor.matmul(br[:, 0:w, :], Fin, TiT, start=False, stop=True)
                nc.tensor.matmul(bi[:, 0:w, :], Fi, TrT, start=True, stop=False)
                nc.tensor.matmul(bi[:, 0:w, :], Fr, TiT, start=False, stop=True)
                gs = slice(g, ge)
                nc.scalar.activation(P4[:, gs, 0:65], br[:, 0:w, :], AF.Copy)
                nc.scalar.activation(P2[:, gs, 0:65], bi[:, 0:w, :], AF.Copy)
                # DC zero == mean subtraction (on SBUF, Pool)
                nc.gpsimd.memset(P4[0:1, gs, 0:1], 0.0)
                nc.gpsimd.memset(P2[0:1, gs, 0:1], 0.0)
                nc.gpsimd.tensor_tensor(P1[:, gs, :], P4[:, gs, :],
                                        P2[:, gs, :], op=ALU.add)
                nc.gpsimd.tensor_tensor(P3[:, gs, :], P2[:, gs, :],
                                        P4[:, gs, :], op=ALU.subtract)

            saved = {}
            pa_tiles = {}

            def stats_part(ci):
                lo, hi = CHUNKS[ci]
                cw = hi - lo
                # reduces on DVE
                nc.vector.tensor_reduce(
                    red[:, lo:hi], Xf[:, lo:hi, :],
                    axis=mybir.AxisListType.X, op=ALU.add)
                sq = sqp.tile([128, 8, W], f32, tag="sq")
                nc.scalar.activation(sq[:, 0:cw, :], Xf[:, lo:hi, :],
                                     AF.Square)
                nc.vector.tensor_reduce(
                    red[:, 32 + lo:32 + hi], sq[:, 0:cw, :],
                    axis=mybir.AxisListType.X, op=ALU.add)
                # partition-sum + scale chain on Pool; Sqrt on Act
                nc.gpsimd.tensor_reduce(
                    stats[0:1, lo:hi], red[:, lo:hi],
                    axis=mybir.AxisListType.C, op=ALU.add)
                nc.gpsimd.tensor_reduce(
                    stats[0:1, 32 + lo:32 + hi], red[:, 32 + lo:32 + hi],
                    axis=mybir.AxisListType.C, op=ALU.add)
                ch = chp.tile([1, 6, 8], f32, tag="ch")
                ssq = ch[:, 0, 0:cw]
                ssqs = ch[:, 1, 0:cw]
                qn = ch[:, 2, 0:cw]
                var = ch[:, 3, 0:cw]
                srt = ch[:, 4, 0:cw]
                tn = ch[:, 5, 0:cw]
                sc2 = chp.tile([1, 4, 8], f32, tag="sc2")
                mask = sc2[:, 3, 0:cw]
                nc.gpsimd.tensor_tensor(ssq, stats[0:1, lo:hi],
                                        stats[0:1, lo:hi], op=ALU.mult)
                nc.gpsimd.tensor_scalar_mul(qn, stats[0:1, 32 + lo:32 + hi],
                                            1.0 / (n - 1.0))
                nc.gpsimd.tensor_scalar_mul(ssqs, ssq,
                                            -1.0 / (n * (n - 1.0)))
                nc.gpsimd.tensor_tensor(var, ssqs, qn, op=ALU.add)
                nc.gpsimd.tensor_scalar(mask, var, STD_EPS * STD_EPS, None,
                                        op0=ALU.is_ge)
                nc.gpsimd.tensor_scalar(tn, var, 1e-30, n,
                                        op0=ALU.max, op1=ALU.mult)
                saved[ci] = (sc2, mask, tn, srt)

            def stage_a_mms(ci):
                lo, hi = CHUNKS[ci]
                for c0 in range(hi - 2, lo - 2, -2):
                    pa = psA.tile([128, 2, 2, 128], f32, tag="pa")
                    pa_tiles[c0] = pa
                    for k in range(2):
                        pav = pa[:, k, :, :].rearrange("p a b -> p (a b)")
                        nc.tensor.matmul(pav, X[:, c0 + k, :], fmov[:, :],
                                         start=True, stop=True)

            def scale_part(ci):
                lo, hi = CHUNKS[ci]
                cw = hi - lo
                sc2, mask, tn, srt = saved.pop(ci)
                # 1/(std*sqrt(n)): DVE reciprocal (one chunk late so tn is
                # long since ready), Act sqrt, Pool mask-mult
                rcp = sc2[:, 2, 0:cw]
                nc.vector.reciprocal(rcp, tn)
                nc.scalar.activation(srt, rcp, AF.Sqrt)
                nc.gpsimd.tensor_tensor(sc2[:, 0, 0:cw], srt, mask,
                                        op=ALU.mult)
                nc.gpsimd.tensor_scalar_mul(sc2[:, 1, 0:cw], sc2[:, 0, 0:cw],
                                            -1.0)
                nc.tensor.matmul(bc_ps[:, lo:hi], ones_row[:, :],
                                 sc2[:, 0, 0:cw], start=True, stop=True)
                nc.tensor.matmul(bc_ps[:, 32 + lo:32 + hi], ones_row[:, :],
                                 sc2[:, 1, 0:cw], start=True, stop=True)
                nc.scalar.copy(bc[:, lo:hi], bc_ps[:, lo:hi])
                nc.scalar.copy(bc[:, 32 + lo:32 + hi],
                               bc_ps[:, 32 + lo:32 + hi])
                for c0 in range(hi - 2, lo - 2, -2):
                    pa = pa_tiles.pop(c0)
                    for k in range(2):
                        c = c0 + k
                        nc.scalar.activation(T_s[:, c, :, 0:65],
                                             pa[:, k, :, 0:65],
                                             AF.Copy, scale=bc[:, c:c + 1])
                for g, ge in GROUPS_AT.get(ci, []):
                    emit_group(g, ge)
                for i in PREFETCH_AT.get(ci, []):
                    get_m(i)

            for ci in range(len(CHUNKS)):
                stats_part(ci)
                if ci > 0:
                    scale_part(ci - 1)
                stage_a_mms(ci)
            scale_part(len(CHUNKS) - 1)

        # =========================== phase 2 ===========================
        # D = S1@m1 + S12@m2 + S2@m3 per pair (contract v); transpose;
        # out = gys-contraction over u; accumulate into out_acc slots.
        with tc.tile_pool(name="psD", bufs=4, space="PSUM") as psD, \
             tc.tile_pool(name="psDT", bufs=2, space="PSUM") as psDT, \
             tc.tile_pool(name="psO", bufs=2, space="PSUM") as psO:

            def emit_bank(bank, nextbank):
                subA, subB = bank
                wA = subA[2]
                nh = 2 if subB is not None else 1
                dps_raw = psD.tile([128, 512], f32, tag="d")  # bank-aligned
                dps = dps_raw[:, 0:455].rearrange("p (a b) -> p a b", a=7)
                for half, sub in ((0, subA), (1, subB)):
                    if sub is None:
                        continue
                    i, s0, w, slot = sub
                    mt = get_m(i)
                    dv = dps[64 * half:64 * half + 64, 0:w, :]
                    for t in range(3):
                        st = smats[:, 64 * t:64 * t + 64]
                        nc.tensor.matmul(
                            dv, st, mt[t][:, s0:s0 + w, 0:65],
                            start=(t == 0), stop=(t == 2),
                            tile_position=(0, 64 * half))
                # prefetch next bank's products so DVE isn't head-of-line
                # blocked behind this bank's PSUM copies
                if nextbank is not None:
                    for sub in nextbank:
                        if sub is not None:
                            get_m(sub[0])
                rows = 128 if nh == 2 else 64
                ds = dspool.tile([128, 7, 65], bf16, tag="ds")
                nc.scalar.activation(ds[0:rows, 0:wA, :], dps[0:rows, 0:wA, :],
                                     AF.Copy)
                # transposes at (0,0): dual reads rows 0:106 (rows 42:64
                # are S-matrix zero pad -> out free 106, not 128); single
                # reads rows 0:42 (out free 42)
                dt_raw = psDT.tile([NU, 1024], bf16, tag="dt")  # bank-aligned
                for t in range(wA):
                    if nh == 2:
                        nc.tensor.transpose(dt_raw[:, 128 * t:128 * t + 106],
                                            ds[0:106, t, :],
                                            id128[0:106, 0:106])
                    else:
                        nc.tensor.transpose(dt_raw[:, 128 * t:128 * t + 42],
                                            ds[0:42, t, :],
                                            id128[0:42, 0:42])
                dt_ps = dt_raw[:, 0:896].rearrange(
                    "p (a b c) -> p a b c", a=7, b=2)
                dt_s = dtpool.tile([NU, 7, 2, 42], bf16, tag="dts")
                nc.vector.tensor_copy(dt_s[:, 0:wA, 0:nh, :],
                                      dt_ps[:, 0:wA, 0:nh, 0:42])

                op_raw = psO.tile([S, 512], f32, tag="ops")  # bank-aligned
                op_ps = op_raw[:, 0:294].rearrange(
                    "p (a b c) -> p a b c", a=2, b=7)
                ov = op_ps[:, 0:nh, 0:wA, :]
                nc.tensor.matmul(ov, gys[:, 0:21],
                                 dt_s[:, 0:wA, 0:nh, 0:21]
                                 .transpose([0, 2, 1, 3]),
                                 start=True, stop=False)
                nc.tensor.matmul(ov, gys[:, 21:42],
                                 dt_s[:, 0:wA, 0:nh, 21:42]
                                 .transpose([0, 2, 1, 3]),
                                 start=False, stop=True)
                # one copy: bank slots are contiguous (A pairs, B pairs)
                slot = subA[3]
                nc.scalar.activation(
                    out_acc[:, slot:slot + nh * wA, :]
                    .rearrange("p (a b) c -> p a b c", a=nh),
                    ov, AF.Copy)

            nbanks = len(banks)
            if DEBUG_STOP == 1:
                nc.vector.memset(out_acc[:, :, :], 0.0)
                nc.vector.tensor_copy(out_acc[:, 0:32, 0:2], P4[0:21, :, 0:2])
                nc.sync.dma_start(out_d, out_acc[:, :, :])
                nbanks = 0
            elif DEBUG_STOP >= 2:
                nbanks = min(nbanks, DEBUG_STOP)
                nc.vector.memset(out_acc[:, :, :], 0.0)
            Q = NPAIR // 8
            dma_done = 0  # slots already sent
            for bi_ in range(nbanks):
                emit_bank(banks[bi_],
                          banks[bi_ + 1] if bi_ + 1 < nbanks else None)
                done_slot = (banks[bi_ + 1][0][3] if bi_ + 1 < nbanks
                             else NPAIR)
                while dma_done < 7 * Q and done_slot >= dma_done + Q:
                    nc.gpsimd.dma_start(
                        out_d[:, dma_done:dma_done + Q, :],
                        out_acc[:, dma_done:dma_done + Q, :])
                    dma_done += Q
            if DEBUG_STOP != 1:
                nc.gpsimd.dma_start(out_d[:, dma_done:, :],
                                    out_acc[:, dma_done:, :])
    nc.compile()
    return nc


_CACHE = {}


def _get_nc():
    if "nc" not in _CACHE:
        _CACHE["nc"] = build_nc()
    return _CACHE["nc"]


TRACE = False


def kernel(x: np.ndarray) -> np.ndarray:
    from concourse.bass_utils import run_bass_kernel_spmd

    assert x.shape == (B, C, H, W) and x.dtype == np.float32
    nc = _get_nc()
    consts = _host_constants()
    in_maps = []
    for b in range(B):
        m = {"x": np.ascontiguousarray(x[b])}
        m.update(consts)
        in_maps.append(m)
    res = run_bass_kernel_spmd(nc, in_maps, core_ids=list(range(B)), trace=TRACE)
    _CACHE["last_results"] = res
    s2p = slot_to_pair()
    out = np.empty((B, NPAIR, S, S), np.float32)
    for b, r in enumerate(res.results):
        o = np.asarray(r["out"])  # [21, 528, 21] (sy, slot, sx)
        out[b, s2p, :, :] = o.transpose(1, 0, 2)
    return out
